# revision 1
# baseline (speedup 1.0000x reference)
"""Trainium2 Bass kernel for the signature-kernel (Goursat PDE) problem.

Full inputs: xs (32, 64, 16) f32, ys (32, 64, 16) f32.
Output: (32, 32) f32 signature-kernel Gram matrix.

Strategy (8 NeuronCores, SPMD, no collectives):
  - Shard batch_x across cores: core c owns a in {4c..4c+3} -> 4*32 = 128
    (x, y) pairs, one pair per SBUF partition.
  - Double increments inc[a,b,i,j] = sum_d Dxs[a,i,d] Dys[b,j,d] are computed
    on-device with 63 PE matmuls using a host-built block-diagonal lhsT so the
    output lands directly in pair-major partition layout.
  - The Goursat PDE recurrence K[i+1,j+1] = c1*(K[i+1,j] + K[i,j+1]) - c2*K[i,j]
    is solved as 126 per-row affine scans x_j = c1_j*x_{j-1} + b_j using the
    DVE TensorTensorScan instruction across all 128 pairs at once (the grid is
    solved transposed - rows=ys-steps - which is valid since the PDE stencil
    is symmetric in (i, j)).
"""

import os
import sys

import numpy as np

for _p in ("/opt/trn_rl_repo", "/root/.axon_site", "/root/.axon_site/_ro/trn_rl_repo",
           "/root/.axon_site/_ro/pypackages"):
    if os.path.isdir(_p) and _p not in sys.path:
        sys.path.append(_p)

_STATE: dict = {}


def _build_program():
    from contextlib import ExitStack

    import concourse.tile as tile
    from concourse import bacc, mybir

    f32 = mybir.dt.float32
    Alu = mybir.AluOpType

    nc = bacc.Bacc(
        "TRN2",
        target_bir_lowering=False,
        debug=False,
        enable_asserts=True,
        num_devices=8,
    )
    bd_d = nc.dram_tensor("bd", [64, 128 * 63], f32, kind="ExternalInput").ap()
    dxs_d = nc.dram_tensor("dxs", [64, 63], f32, kind="ExternalInput").ap()
    out_d = nc.dram_tensor("out", [128, 1], f32, kind="ExternalOutput").ap()

    with ExitStack() as ctx:
        tc = ctx.enter_context(tile.TileContext(nc))
        ws = ctx.enter_context(tc.tile_pool(name="ws", bufs=1))
        pp = ctx.enter_context(tc.tile_pool(name="pp", bufs=1, space="PSUM"))
        tmp = ctx.enter_context(tc.tile_pool(name="tmp", bufs=2))

        bd_sb = ws.tile([64, 128 * 63], f32)
        nc.sync.dma_start(out=bd_sb[:], in_=bd_d)
        dxs_sb = ws.tile([64, 63], f32)
        nc.sync.dma_start(out=dxs_sb[:], in_=dxs_d)

        # inc[(a,b), i] for each ys-step j: out = bd[:, :, j].T @ dxs
        ps = pp.tile([128, 63, 64], f32)  # strip j at [:, j, 0:63]; 256B stride
        bd_v = bd_sb[:].rearrange("k (p j) -> k p j", j=63)
        for j in range(63):
            nc.tensor.matmul(
                ps[:, j, 0:63], bd_v[:, :, j], dxs_sb[:], start=True, stop=True
            )

        # vf[p, j*63 + i] (grid rows = ys-steps)
        vf = ws.tile([128, 3969], f32)
        nc.scalar.copy(vf[:], ps[:, :, 0:63])

        # c1 = 1 + vf/2 + vf^2/12 ; c2 = 1 - vf^2/12  (half-resolution)
        sq = ws.tile([128, 3969], f32)
        nc.vector.tensor_mul(sq[:], vf[:], vf[:])
        c2h = ws.tile([128, 3969], f32)
        nc.vector.tensor_scalar(
            out=c2h[:], in0=sq[:], scalar1=-1.0 / 12.0, scalar2=1.0,
            op0=Alu.mult, op1=Alu.add,
        )
        u1 = ws.tile([128, 3969], f32)
        nc.vector.tensor_scalar(
            out=u1[:], in0=sq[:], scalar1=1.0 / 12.0, scalar2=1.0,
            op0=Alu.mult, op1=Alu.add,
        )
        c1h = ws.tile([128, 3969], f32)
        nc.vector.scalar_tensor_tensor(
            c1h[:], vf[:], 0.5, u1[:], Alu.mult, Alu.add
        )

        # expand columns 2x (dyadic refinement) to full-width coefficient rows
        c1f = ws.tile([128, 63, 126], f32)
        c2f = ws.tile([128, 63, 126], f32)
        c1h_dup = (
            c1h[:].rearrange("p (h m) -> p h m", h=63)
            .unsqueeze(3).broadcast_to((128, 63, 63, 2))
        )
        c2h_dup = (
            c2h[:].rearrange("p (h m) -> p h m", h=63)
            .unsqueeze(3).broadcast_to((128, 63, 63, 2))
        )
        nc.scalar.copy(c1f[:], c1h_dup)
        nc.gpsimd.tensor_copy(c2f[:], c2h_dup)

        # Goursat row recurrence; K rows double-buffered, col 0 always 1
        kb = ws.tile([128, 2, 127], f32)
        nc.vector.memset(kb[:, 0, :], 1.0)
        nc.vector.memset(kb[:, 1, 0:1], 1.0)

        for r in range(126):
            h = r >> 1
            pr = r & 1
            nx = 1 - pr
            t1 = tmp.tile([128, 126], f32, tag="t1")
            nc.vector.tensor_mul(t1[:], c1f[:, h, :], kb[:, pr, 1:127])
            u = tmp.tile([128, 126], f32, tag="u")
            nc.vector.scalar_tensor_tensor(
                u[:], kb[:, pr, 0:126], -1.0, c2f[:, h, :], Alu.mult, Alu.mult
            )
            b = tmp.tile([128, 126], f32, tag="b")
            nc.vector.tensor_add(b[:], t1[:], u[:])
            nc.vector.tensor_tensor_scan(
                kb[:, nx, 1:127], c1f[:, h, :], b[:], 1.0, Alu.mult, Alu.add
            )

        nc.sync.dma_start(out=out_d, in_=kb[:, 0, 126:127])

    nc.compile()
    return nc


def _get_nc():
    if "nc" not in _STATE:
        _STATE["nc"] = _build_program()
    return _STATE["nc"]


def _make_inputs(xs: np.ndarray, ys: np.ndarray):
    xs = np.asarray(xs, dtype=np.float32)
    ys = np.asarray(ys, dtype=np.float32)
    dxs_all = (xs[:, 1:, :] - xs[:, :-1, :]) * np.float32(0.25)  # (32, 63, 16)
    dys = ys[:, 1:, :] - ys[:, :-1, :]                           # (32, 63, 16)

    dysT = np.ascontiguousarray(dys.transpose(2, 0, 1))          # [d, b, j]
    bd = np.zeros((4, 16, 4, 32, 63), np.float32)
    for g in range(4):
        bd[g, :, g] = dysT
    bd = np.ascontiguousarray(bd.reshape(64, 128 * 63))

    in_maps = []
    for c in range(8):
        dxs_c = np.ascontiguousarray(
            dxs_all[4 * c : 4 * c + 4].transpose(0, 2, 1).reshape(64, 63)
        )  # [(a'*16+d), i]
        in_maps.append({"bd": bd, "dxs": dxs_c})
    return in_maps


def _run(nc, in_maps, **kwargs):
    from concourse.bass_utils import run_bass_kernel_spmd

    return run_bass_kernel_spmd(nc, in_maps, list(range(8)), **kwargs)


def kernel(xs: np.ndarray, ys: np.ndarray) -> np.ndarray:
    nc = _get_nc()
    in_maps = _make_inputs(xs, ys)
    res = _run(nc, in_maps)
    out = np.concatenate(
        [np.asarray(res.results[c]["out"]).reshape(4, 32) for c in range(8)], axis=0
    )
    return out.astype(np.float32)


# revision 2
# speedup vs baseline: 1.2277x; 1.2277x over previous
"""Trainium2 Bass kernel for the signature-kernel (Goursat PDE) problem.

Full inputs: xs (32, 64, 16) f32, ys (32, 64, 16) f32.
Output: (32, 32) f32 signature-kernel Gram matrix.

Strategy (8 NeuronCores, SPMD, no collectives):
  - Shard batch_x across cores: core c owns a in {4c..4c+3} -> 4*32 = 128
    (x, y) pairs, one pair per SBUF partition.
  - Double increments inc[a,b,i,j] = sum_d Dxs[a,i,d] Dys[b,j,d] are computed
    on-device with 63 PE matmuls using a host-built block-diagonal lhsT
    (contraction over (a', d), a'-blocks of Dys) so the output lands directly
    in pair-major partition layout. The producer pipeline (DMA, matmul, PSUM
    copy, coefficient build, dyadic column expansion) is chunked along j so
    the PDE row loop starts after the first chunk.
  - The Goursat PDE recurrence K[i+1,j+1] = c1*(K[i+1,j] + K[i,j+1]) - c2*K[i,j]
    is solved as 126 per-row affine scans x_j = c1_j*x_{j-1} + b_j using the
    DVE TensorTensorScan instruction across all 128 pairs at once (the grid is
    solved transposed - rows=ys-steps - which is valid since the PDE stencil
    is symmetric in (i, j)).
"""

import os
import sys

import numpy as np

for _p in ("/opt/trn_rl_repo", "/root/.axon_site", "/root/.axon_site/_ro/trn_rl_repo",
           "/root/.axon_site/_ro/pypackages"):
    if os.path.isdir(_p) and _p not in sys.path:
        sys.path.append(_p)

_STATE: dict = {}

NCHUNK = 4
JCH = [(16, 0), (16, 16), (16, 32), (15, 48)]  # (len, start) j-chunks of 63


def _build_program():
    from contextlib import ExitStack

    import concourse.tile as tile
    from concourse import bacc, mybir

    f32 = mybir.dt.float32
    Alu = mybir.AluOpType

    nc = bacc.Bacc(
        "TRN2",
        target_bir_lowering=False,
        debug=False,
        enable_asserts=True,
        num_devices=8,
    )
    # bd[(a'*16+d), j, (a*32+b)] = delta_{a,a'} * Dys[b, j, d]
    bd_d = nc.dram_tensor("bd", [64, 63 * 128], f32, kind="ExternalInput").ap()
    dxs_d = nc.dram_tensor("dxs", [64, 63], f32, kind="ExternalInput").ap()
    out_d = nc.dram_tensor("out", [128, 1], f32, kind="ExternalOutput").ap()

    with ExitStack() as ctx:
        tc = ctx.enter_context(tile.TileContext(nc))
        ws = ctx.enter_context(tc.tile_pool(name="ws", bufs=1))
        pp = ctx.enter_context(tc.tile_pool(name="pp", bufs=1, space="PSUM"))
        tmp = ctx.enter_context(tc.tile_pool(name="tmp", bufs=2))

        dxs_sb = ws.tile([64, 63], f32)
        nc.sync.dma_start(out=dxs_sb[:], in_=dxs_d)
        bd_sb = ws.tile([64, 63, 128], f32)
        bd_v = bd_d.rearrange("k (j p) -> k j p", j=63)
        for ln, st in JCH:
            nc.sync.dma_start(
                out=bd_sb[:, st : st + ln, :], in_=bd_v[:, st : st + ln, :]
            )

        # K rows double-buffered, col 0 always 1
        kb = ws.tile([128, 2, 127], f32)
        nc.vector.memset(kb[:, 0, :], 1.0)
        nc.vector.memset(kb[:, 1, 0:1], 1.0)

        ps = pp.tile([128, 63, 64], f32)  # strip j at [:, j, 0:63]; 256B stride
        vf = ws.tile([128, 63, 63], f32)  # vf[p, j, i] (grid rows = ys-steps)
        sq = ws.tile([128, 63, 63], f32)
        m2h = ws.tile([128, 63, 63], f32)   # -c2 = vf^2/12 - 1 (half-res)
        c1hm2 = ws.tile([128, 63, 63], f32)  # c1 - 2 = vf/2 + vf^2/12
        c1f = ws.tile([128, 63, 126], f32)   # c1, full-width rows (for scan)

        for ln, st in JCH:
            jsl = slice(st, st + ln)
            for j in range(st, st + ln):
                nc.tensor.matmul(
                    ps[:, j, 0:63], bd_sb[:, j, :], dxs_sb[:], start=True, stop=True
                )
            nc.scalar.copy(vf[:, jsl, :], ps[:, jsl, 0:63])
            nc.vector.tensor_mul(sq[:, jsl, :], vf[:, jsl, :], vf[:, jsl, :])
            nc.vector.tensor_scalar(
                out=m2h[:, jsl, :], in0=sq[:, jsl, :],
                scalar1=1.0 / 12.0, scalar2=-1.0, op0=Alu.mult, op1=Alu.add,
            )
            nc.vector.scalar_tensor_tensor(
                c1hm2[:, jsl, :], vf[:, jsl, :], 0.5, m2h[:, jsl, :],
                Alu.mult, Alu.add,
            )
            # expand columns 2x and add back the +2: c1 = (c1 - 2) + 2
            dup = c1hm2[:, jsl, :].unsqueeze(3).broadcast_to((128, ln, 63, 2))
            nc.scalar.activation(
                out=c1f[:, jsl, :], in_=dup,
                func=mybir.ActivationFunctionType.Copy, bias=2.0, scale=1.0,
            )

        for r in range(126):
            h = r >> 1
            pr = r & 1
            nx = 1 - pr
            m2row = m2h[:, h, :].unsqueeze(2).broadcast_to((128, 63, 2))
            t1 = tmp.tile([128, 126], f32, tag="t1")
            nc.vector.tensor_mul(t1[:], c1f[:, h, :], kb[:, pr, 1:127])
            u = tmp.tile([128, 126], f32, tag="u")
            nc.vector.tensor_mul(u[:], kb[:, pr, 0:126], m2row)
            b = tmp.tile([128, 126], f32, tag="b")
            nc.vector.tensor_add(b[:], t1[:], u[:])
            nc.vector.tensor_tensor_scan(
                kb[:, nx, 1:127], c1f[:, h, :], b[:], 1.0, Alu.mult, Alu.add
            )

        nc.sync.dma_start(out=out_d, in_=kb[:, 0, 126:127])

    nc.compile()
    return nc


def _get_nc():
    if "nc" not in _STATE:
        _STATE["nc"] = _build_program()
    return _STATE["nc"]


def _make_inputs(xs: np.ndarray, ys: np.ndarray):
    xs = np.asarray(xs, dtype=np.float32)
    ys = np.asarray(ys, dtype=np.float32)
    dxs_all = (xs[:, 1:, :] - xs[:, :-1, :]) * np.float32(0.25)  # (32, 63, 16)
    dys = ys[:, 1:, :] - ys[:, :-1, :]                           # (32, 63, 16)

    dysT = np.ascontiguousarray(dys.transpose(2, 1, 0))          # [d, j, b]
    bd = np.zeros((4, 16, 63, 4, 32), np.float32)
    for g in range(4):
        bd[g, :, :, g, :] = dysT
    bd = np.ascontiguousarray(bd.reshape(64, 63 * 128))

    in_maps = []
    for c in range(8):
        dxs_c = np.ascontiguousarray(
            dxs_all[4 * c : 4 * c + 4].transpose(0, 2, 1).reshape(64, 63)
        )  # [(a'*16+d), i]
        in_maps.append({"bd": bd, "dxs": dxs_c})
    return in_maps


def _run(nc, in_maps, **kwargs):
    from concourse.bass_utils import run_bass_kernel_spmd

    return run_bass_kernel_spmd(nc, in_maps, list(range(8)), **kwargs)


def kernel(xs: np.ndarray, ys: np.ndarray) -> np.ndarray:
    nc = _get_nc()
    in_maps = _make_inputs(xs, ys)
    res = _run(nc, in_maps)
    out = np.concatenate(
        [np.asarray(res.results[c]["out"]).reshape(4, 32) for c in range(8)], axis=0
    )
    return out.astype(np.float32)


# revision 6
# speedup vs baseline: 1.3727x; 1.1181x over previous
"""Trainium2 Bass kernel for the signature-kernel (Goursat PDE) problem.

Full inputs: xs (32, 64, 16) f32, ys (32, 64, 16) f32.
Output: (32, 32) f32 signature-kernel Gram matrix.

Strategy (8 NeuronCores, SPMD, no collectives):
  - Shard batch_x across cores: core c owns a in {4c..4c+3} -> 4*32 = 128
    (x, y) pairs, one pair per SBUF partition.
  - Double increments inc[a,b,i,j] = sum_d Dxs[a,i,d] Dys[b,j,d] are computed
    on-device with 63 PE matmuls using a host-built block-diagonal lhsT
    (contraction over (a', d), a'-blocks of Dys) so the output lands directly
    in pair-major partition layout. The producer pipeline (DMA, matmul, PSUM
    copy, coefficient build, dyadic column expansion) is chunked along j so
    the PDE row loop starts after the first chunk.
  - The Goursat PDE recurrence K[i+1,j+1] = c1*(K[i+1,j] + K[i,j+1]) - c2*K[i,j]
    is solved as 126 per-row affine scans x_j = c1_j*x_{j-1} + b_j using the
    DVE TensorTensorScan instruction across all 128 pairs at once (the grid is
    solved transposed - rows=ys-steps - which is valid since the PDE stencil
    is symmetric in (i, j)).
"""

import os
import sys

import numpy as np

for _p in ("/opt/trn_rl_repo", "/root/.axon_site", "/root/.axon_site/_ro/trn_rl_repo",
           "/root/.axon_site/_ro/pypackages"):
    if os.path.isdir(_p) and _p not in sys.path:
        sys.path.append(_p)

_STATE: dict = {}

JCH = [(8, 0), (8, 8), (8, 16), (8, 24), (8, 32), (8, 40), (8, 48), (7, 56)]
BF16_MM = os.environ.get("SIG_BF16_MM", "0") == "1"


def _build_program():
    from contextlib import ExitStack

    import concourse.tile as tile
    from concourse import bacc, mybir

    f32 = mybir.dt.float32
    Alu = mybir.AluOpType

    nc = bacc.Bacc(
        "TRN2",
        target_bir_lowering=False,
        debug=False,
        enable_asserts=True,
        num_devices=8,
    )
    in_dt = mybir.dt.bfloat16 if BF16_MM else f32
    # bd[(a'*16+d), j, (a*32+b)] = delta_{a,a'} * Dys[b, j, d]
    bd_d = nc.dram_tensor("bd", [64, 63 * 128], in_dt, kind="ExternalInput").ap()
    dxs_d = nc.dram_tensor("dxs", [64, 63], in_dt, kind="ExternalInput").ap()
    out_d = nc.dram_tensor("out", [128, 1], f32, kind="ExternalOutput").ap()

    with ExitStack() as ctx:
        tc = ctx.enter_context(tile.TileContext(nc))
        ws = ctx.enter_context(tc.tile_pool(name="ws", bufs=1))
        pp = ctx.enter_context(tc.tile_pool(name="pp", bufs=1, space="PSUM"))
        tmp = ctx.enter_context(tc.tile_pool(name="tmp", bufs=2))

        dxs_sb = ws.tile([64, 63], in_dt)
        nc.sync.dma_start(out=dxs_sb[:], in_=dxs_d)
        bd_sb = ws.tile([64, 63, 128], in_dt)
        bd_v = bd_d.rearrange("k (j p) -> k j p", j=63)
        for ln, st in JCH:
            nc.sync.dma_start(
                out=bd_sb[:, st : st + ln, :], in_=bd_v[:, st : st + ln, :]
            )

        # K rows double-buffered, col 0 always 1
        kb = ws.tile([128, 2, 127], f32)
        nc.vector.memset(kb[:, 0, :], 1.0)
        nc.vector.memset(kb[:, 1, 0:1], 1.0)

        ps = pp.tile([128, 63, 64], f32)  # strip j at [:, j, 0:63]; 256B stride
        vf = ws.tile([128, 63, 63], f32)  # vf[p, j, i] (grid rows = ys-steps)
        sq = ws.tile([128, 63, 63], f32)
        m2h = ws.tile([128, 63, 63], f32)   # -c2 = vf^2/12 - 1 (half-res)
        c1hm2 = ws.tile([128, 63, 63], f32)  # c1 - 2 = vf/2 + vf^2/12
        c1f = ws.tile([128, 63, 126], f32)   # c1, full-width rows (for scan)

        for ln, st in JCH:
            jsl = slice(st, st + ln)
            for j in range(st, st + ln):
                nc.tensor.matmul(
                    ps[:, j, 0:63], bd_sb[:, j, :], dxs_sb[:], start=True, stop=True
                )
            nc.scalar.copy(vf[:, jsl, :], ps[:, jsl, 0:63])
            nc.vector.tensor_mul(sq[:, jsl, :], vf[:, jsl, :], vf[:, jsl, :])
            nc.vector.tensor_scalar(
                out=m2h[:, jsl, :], in0=sq[:, jsl, :],
                scalar1=1.0 / 12.0, scalar2=-1.0, op0=Alu.mult, op1=Alu.add,
            )
            nc.vector.scalar_tensor_tensor(
                c1hm2[:, jsl, :], vf[:, jsl, :], 0.5, m2h[:, jsl, :],
                Alu.mult, Alu.add,
            )
            # expand columns 2x and add back the +2: c1 = (c1 - 2) + 2
            dup = c1hm2[:, jsl, :].unsqueeze(3).broadcast_to((128, ln, 63, 2))
            nc.scalar.activation(
                out=c1f[:, jsl, :], in_=dup,
                func=mybir.ActivationFunctionType.Copy, bias=2.0, scale=1.0,
            )

        for r in range(126):
            h = r >> 1
            pr = r & 1
            nx = 1 - pr
            m2row = m2h[:, h, :].unsqueeze(2).broadcast_to((128, 63, 2))
            t1 = tmp.tile([128, 126], f32, tag="t1")
            nc.vector.tensor_mul(t1[:], c1f[:, h, :], kb[:, pr, 1:127])
            u = tmp.tile([128, 126], f32, tag="u")
            nc.vector.tensor_mul(u[:], kb[:, pr, 0:126], m2row)
            b = tmp.tile([128, 126], f32, tag="b")
            nc.vector.tensor_add(b[:], t1[:], u[:])
            nc.vector.tensor_tensor_scan(
                kb[:, nx, 1:127], c1f[:, h, :], b[:], 1.0, Alu.mult, Alu.add
            )

        nc.sync.dma_start(out=out_d, in_=kb[:, 0, 126:127])

    nc.compile()
    return nc


def _get_nc():
    if "nc" not in _STATE:
        _STATE["nc"] = _build_program()
    return _STATE["nc"]


def _make_inputs(xs: np.ndarray, ys: np.ndarray):
    xs = np.asarray(xs, dtype=np.float32)
    ys = np.asarray(ys, dtype=np.float32)
    dxs_all = (xs[:, 1:, :] - xs[:, :-1, :]) * np.float32(0.25)  # (32, 63, 16)
    dys = ys[:, 1:, :] - ys[:, :-1, :]                           # (32, 63, 16)

    dysT = np.ascontiguousarray(dys.transpose(2, 1, 0))          # [d, j, b]
    bd = np.zeros((4, 16, 63, 4, 32), np.float32)
    for g in range(4):
        bd[g, :, :, g, :] = dysT
    bd = np.ascontiguousarray(bd.reshape(64, 63 * 128))

    if BF16_MM:
        import ml_dtypes

        bd = bd.astype(ml_dtypes.bfloat16)

    in_maps = []
    for c in range(8):
        dxs_c = np.ascontiguousarray(
            dxs_all[4 * c : 4 * c + 4].transpose(0, 2, 1).reshape(64, 63)
        )  # [(a'*16+d), i]
        if BF16_MM:
            import ml_dtypes

            dxs_c = dxs_c.astype(ml_dtypes.bfloat16)
        in_maps.append({"bd": bd, "dxs": dxs_c})
    return in_maps


def _run(nc, in_maps, **kwargs):
    from concourse.bass_utils import run_bass_kernel_spmd

    return run_bass_kernel_spmd(nc, in_maps, list(range(8)), **kwargs)


def kernel(xs: np.ndarray, ys: np.ndarray) -> np.ndarray:
    nc = _get_nc()
    in_maps = _make_inputs(xs, ys)
    res = _run(nc, in_maps)
    out = np.concatenate(
        [np.asarray(res.results[c]["out"]).reshape(4, 32) for c in range(8)], axis=0
    )
    return out.astype(np.float32)
